# revision 28
# baseline (speedup 1.0000x reference)
"""Multi-head attention (B=4, T=2048, H=8, D=64, C=512) on 8 NeuronCores.

Sharding: core c -> batch b = c//2, head group c%2 (4 heads = 256 inner rows).
Each core computes attention for its 4 heads plus the *partial* output
projection over its 256 inner rows; the host sums the two partials per batch
(the unshard of the contraction-sharded inner dimension).

Per-core dataflow (all fp32):
  qT,kT [128, 2, T] SBUF   pair-packed: partition = (h%2)*64 + d, dim1 = h//2
  v     [128, 16, 4, 128]  natural [keys, *] per head; 64 v-columns plus a
                           64-wide ones block. Even heads: [v | ones]; odd
                           heads: [ones | v]. The PV matmul then yields the
                           attention output on the partitions where the final
                           projection wants it, and the softmax denominator
                           broadcast over the other 64 partitions -- both for
                           free, since matmul cost is N (stream) cycles.
  scores: S^T per (pair, i-chunk, key-block) -> PSUM [128,512]; the two heads
          of a pair run as concurrent 64-row PE tiles (auto tile_position from
          base partitions 0/64). exp on ScalarE evacuates PSUM -> panel.
  PV:     po[128,512] accumulated over 16 key-blocks; recip of the denominator
          half, SBUF->SBUF DMA to the numerator partitions (DVE has no
          cross-lane path), then one fused multiply evacuates + normalizes.
"""

import numpy as np

B, T, C = 4, 2048, 512
H, D = 8, 64
INNER = H * D  # 512
O = 512
P = 128
IC = 512            # i-chunk (query) width
N_IC = T // IC      # 4
N_JB = T // P       # 16 key blocks
JBH = N_JB // 2     # key blocks per panel half
KC = C // P         # 4 contraction chunks
RL = 256            # local inner rows per core (4 heads)

_compiled = None
_last_in_maps = None


def _build():
    import concourse.bacc as bacc
    import concourse.tile as tile
    import concourse.mybir as mybir

    F32 = mybir.dt.float32
    F32R = mybir.dt.float32r
    Exp = mybir.ActivationFunctionType.Exp

    nc = bacc.Bacc("TRN2", target_bir_lowering=False, debug=False, num_devices=8)

    xT_d = nc.dram_tensor("xT", [C, T], F32R, kind="ExternalInput")
    wqT_d = nc.dram_tensor("wqT", [C, RL], F32R, kind="ExternalInput")
    wkT_d = nc.dram_tensor("wkT", [C, RL], F32R, kind="ExternalInput")
    wvT_d = nc.dram_tensor("wvT", [C, RL], F32R, kind="ExternalInput")
    wpT_d = nc.dram_tensor("wpT", [RL, O], F32R, kind="ExternalInput")
    out_d = nc.dram_tensor("out", [T, O], F32, kind="ExternalOutput")

    JBQ = 4  # key-blocks per panel quarter

    with tile.TileContext(nc) as tc:
        with (
            tc.tile_pool(name="persist", bufs=1) as persist,
            tc.tile_pool(name="rcp", bufs=2) as rcpool,
            tc.tile_pool(name="evac", bufs=2) as evac,
        ):
            wq = persist.tile([P, KC, RL], F32R, tag="wq")
            wk = persist.tile([P, KC, RL], F32R, tag="wk")
            wv = persist.tile([P, KC, RL], F32R, tag="wv")
            wp = persist.tile([P, 2, O], F32R, tag="wp")
            nc.sync.dma_start(wk[:], wkT_d.ap().rearrange("(kc p) r -> p kc r", p=P))

            qT = persist.tile([P, 2, T], F32R, tag="qT")
            kT = persist.tile([P, 2, T], F32R, tag="kT")
            vsb = persist.tile([P, N_JB, 4, P], F32R, tag="v")
            outT = persist.tile([P, 2, T], F32R, tag="outT")

            with tc.tile_pool(name="xtp", bufs=1) as xtp:
                # chunked xT load so the K projection starts on chunk 0
                xt = xtp.tile([P, KC, T], F32R, tag="xt")
                xt_r = xT_d.ap().rearrange("(kc p) t -> p kc t", p=P)
                for kc in range(KC - 1):
                    nc.sync.dma_start(xt[:, kc, :], xt_r[:, kc, :])
                nc.sync.dma_start(
                    wq[:], wqT_d.ap().rearrange("(kc p) r -> p kc r", p=P)
                )
                nc.sync.dma_start(xt[:, KC - 1, :], xt_r[:, KC - 1, :])
                nc.sync.dma_start(
                    wv[:], wvT_d.ap().rearrange("(kc p) r -> p kc r", p=P)
                )
                nc.sync.dma_start(
                    wp[:], wpT_d.ap().rearrange("(rc p) o -> p rc o", p=P)
                )

                # prewarm the exp table while DMAs stream
                warm = xtp.tile([P, 8], F32, tag="warm")
                nc.vector.memset(warm[:], 0.0)
                nc.scalar.activation(warm[:], warm[:], Exp)

                # keep the PE busy during the xT load so it is at full clock
                # (and past the HAM ramp) when the projections start
                wmm0 = xtp.tile([P, 256], F32, tag="wmm0")
                nc.vector.memset(wmm0[:], 0.0)
                wmm = xtp.tile([P, 256], F32R, tag="wmm")
                nc.vector.tensor_copy(wmm[:], wmm0[:])

                # ---- K projection, kc-outer: 8 live PSUM groups ----
                with tc.tile_pool(name="pk", bufs=5, space="PSUM") as pk:
                    wps = pk.tile([P, IC], F32, tag="kp", name="wps")
                    # only the first head-pair's K rows upfront (chunk order
                    # is pair-major, so rows 128:256 aren't needed until the
                    # 5th chunk and are projected inside chunk 1's PV loop)
                    kps = [
                        pk.tile([P, IC], F32, tag="kp", name=f"kp{g}")
                        for g in range(N_IC)
                    ]
                    for kc in range(KC):
                        for _ in range(20 if kc < KC - 1 else 0):
                            nc.tensor.matmul(wps[:, 0:256], lhsT=wmm[:, 0:P],
                                             rhs=wmm[:], start=True, stop=True)
                        for ic in range(N_IC):
                            nc.tensor.matmul(
                                kps[ic][:],
                                lhsT=wk[:, kc, 0:P],
                                rhs=xt[:, kc, ic * IC:(ic + 1) * IC],
                                start=(kc == 0), stop=(kc == KC - 1),
                            )
                    # evac the block chunk-0 scores need first, then squeeze
                    # chunk-0's Q projection in before the remaining evacs so
                    # the first exp isn't stuck behind them in the DVE queue
                    nc.vector.tensor_copy(kT[:, 0, 0:IC], kps[0][:])
                    qp0 = pk.tile([P, IC], F32, tag="kp", name="qp0")
                    for kc in range(KC):
                        nc.tensor.matmul(
                            qp0[:],
                            lhsT=wq[:, kc, 0:P],
                            rhs=xt[:, kc, 0:IC],
                            start=(kc == 0), stop=(kc == KC - 1),
                        )
                    nc.vector.tensor_copy(qT[:, 0, 0:IC], qp0[:])
                    for ic in range(1, N_IC):
                        nc.vector.tensor_copy(
                            kT[:, 0, ic * IC:(ic + 1) * IC], kps[ic][:]
                        )

                with (
                    tc.tile_pool(name="psA", bufs=2, space="PSUM") as psA,
                    tc.tile_pool(name="psB", bufs=4, space="PSUM") as psB,
                ):
                    # ones blocks of v (V projection itself is emitted after
                    # the first scores batch so ACT starts as early as possible)
                    ones_st = xtp.tile([P, N_JB, 64], F32, tag="ones")
                    nc.vector.memset(ones_st[:], 1.0)
                    ones_b = ones_st[:, :, None, :].to_broadcast(
                        (P, N_JB, 2, 64)
                    )
                    nc.vector.tensor_copy(vsb[:, :, 0:2, 64:128], ones_b)
                    nc.vector.tensor_copy(vsb[:, :, 2:4, 0:64], ones_b)

                    def k_rb1_group(ic):
                        kp = psB.tile([P, IC], F32, tag="o", name="kp1")
                        for kc in range(KC):
                            nc.tensor.matmul(
                                kp[:],
                                lhsT=wk[:, kc, P:2 * P],
                                rhs=xt[:, kc, ic * IC:(ic + 1) * IC],
                                start=(kc == 0), stop=(kc == KC - 1),
                            )
                        nc.vector.tensor_copy(
                            kT[:, 1, ic * IC:(ic + 1) * IC], kp[:]
                        )

                    def v_proj_group(jb):
                        ps = psB.tile([P, IC], F32, tag="o", name="vp")
                        for kc in range(KC):
                            nc.tensor.matmul(
                                ps[:, :RL],
                                lhsT=xt[:, kc, jb * P:(jb + 1) * P],
                                rhs=wv[:, kc, :],
                                start=(kc == 0), stop=(kc == KC - 1),
                            )
                        ps_r = ps[:, 0:RL].rearrange(
                            "p (e o d) -> p o e d", e=2, o=2
                        )
                        nc.vector.tensor_copy(vsb[:, jb, 0:2, 0:64],
                                              ps_r[:, 0])
                        nc.vector.tensor_copy(vsb[:, jb, 2:4, 64:128],
                                              ps_r[:, 1])

                    # ---- attention, i-chunk outer; final projection inline ----
                    # Pipelined emission: chunk c+1's scores quarters are
                    # interleaved into chunk c's PV segments (each quarter's
                    # panel slot is freed by the PV segment emitted just
                    # before it), so ScalarE never starves between chunks.
                    def q_proj(ic, pair):
                        icsl = slice(ic * IC, (ic + 1) * IC)
                        qp = psB.tile([P, IC], F32, tag="o", name="qp")
                        for kc in range(KC):
                            nc.tensor.matmul(
                                qp[:],
                                lhsT=wq[:, kc, pair * P:(pair + 1) * P],
                                rhs=xt[:, kc, icsl],
                                start=(kc == 0), stop=(kc == KC - 1),
                            )
                        nc.vector.tensor_copy(qT[:, pair, icsl], qp[:])

                    with tc.tile_pool(name="panels", bufs=4) as panels:
                        chunks = [(ic, pair) for pair in range(2)
                                  for ic in range(N_IC)]
                        NQ = N_JB // JBQ  # quarters per chunk
                        state = {}

                        def scores_quarter(idx):
                            ic, pair = chunks[idx]
                            icsl = slice(ic * IC, (ic + 1) * IC)
                            st = state.setdefault(idx, [])
                            quarter = len(st)
                            pan = panels.tile([P, JBQ, 2, IC], F32R, tag="pan")
                            st.append(pan)
                            for j4 in range(JBQ):
                                jb = quarter * JBQ + j4
                                ps = psA.tile([P, 2, IC], F32, tag="s")
                                for hh in range(2):
                                    nc.tensor.matmul(
                                        ps[:, hh, :],
                                        lhsT=kT[hh * 64:(hh + 1) * 64, pair,
                                                jb * P:(jb + 1) * P],
                                        rhs=qT[hh * 64:(hh + 1) * 64, pair,
                                               icsl],
                                        start=True, stop=True,
                                        skip_group_check=True,
                                    )
                                nc.scalar.activation(pan[:, j4, :, :], ps[:],
                                                     Exp)

                        for quarter in range(NQ):
                            scores_quarter(0)

                        for idx, (ic, pair) in enumerate(chunks):
                            icsl = slice(ic * IC, (ic + 1) * IC)
                            quarters = state[idx]
                            pos = []
                            for hh in range(2):
                                po = psB.tile([P, IC], F32, tag="o",
                                              name=f"po{hh}")
                                pos.append(po)
                            if idx + 1 < len(chunks):
                                q_proj(*chunks[idx + 1])
                            for quarter in range(NQ):
                                if idx == 1:
                                    k_rb1_group(quarter)
                                for j4 in range(JBQ):
                                    jb = quarter * JBQ + j4
                                    if idx == 0:
                                        v_proj_group(jb)
                                    for hh in range(2):
                                        nc.tensor.matmul(
                                            pos[hh][:],
                                            lhsT=vsb[:, jb, hh * 2 + pair, :],
                                            rhs=quarters[quarter][:, j4, hh, :],
                                            start=(jb == 0),
                                            stop=(jb == N_JB - 1),
                                        )
                                # chunk idx+1's scores quarter reuses the
                                # panel slot the PV segment above released
                                if idx + 1 < len(chunks):
                                    scores_quarter(idx + 1)
                            sls = [(slice(0, 64), slice(64, 128)),
                                   (slice(64, 128), slice(0, 64))]
                            rcs = []
                            for hh in range(2):
                                num_sl, den_sl = sls[hh]
                                rc = rcpool.tile([P, IC], F32, tag="rc")
                                rcs.append(rc)
                                nc.vector.reciprocal(rc[den_sl, :],
                                                     pos[hh][den_sl, :])
                            for hh in range(2):
                                num_sl, den_sl = sls[hh]
                                nc.sync.dma_start(rcs[hh][num_sl, :],
                                                  rcs[hh][den_sl, :])
                            for hh in range(2):
                                num_sl, den_sl = sls[hh]
                                nc.vector.tensor_mul(
                                    outT[num_sl, pair, icsl],
                                    pos[hh][num_sl, :],
                                    rcs[hh][num_sl, :],
                                )

                            if pair == 1:
                                # output projection for this chunk's i-blocks
                                for ib in range(ic * IC // P,
                                                (ic + 1) * IC // P):
                                    fp = psB.tile([P, IC], F32, tag="o",
                                                  name="fp")
                                    for pr in range(2):
                                        nc.tensor.matmul(
                                            fp[:],
                                            lhsT=outT[:, pr,
                                                      ib * P:(ib + 1) * P],
                                            rhs=wp[:, pr, :],
                                            start=(pr == 0), stop=(pr == 1),
                                        )
                                    ev = evac.tile([P, O], F32, tag="ev")
                                    if ic == N_IC - 1 and ib % 2 == 0:
                                        nc.scalar.copy(ev[:], fp[:])
                                    else:
                                        nc.vector.tensor_copy(ev[:], fp[:])
                                    nc.sync.dma_start(
                                        out_d[ib * P:(ib + 1) * P, :], ev[:]
                                    )

    nc.compile()
    return nc


def _get_compiled():
    global _compiled
    if _compiled is None:
        _compiled = _build()
    return _compiled


def _round_f32r(a):
    """Round fp32 to the FP32R-representable set: exact bf16 hi + bf16 lo."""
    import ml_dtypes

    a = np.asarray(a, dtype=np.float32)
    hi = a.astype(ml_dtypes.bfloat16).astype(np.float32)
    lo = (a - hi).astype(ml_dtypes.bfloat16).astype(np.float32)
    return hi + lo


def kernel(x, Wk, Wq, Wv, Wp, causal_mask):
    from concourse.bass_utils import run_bass_kernel_spmd

    assert not int(np.asarray(causal_mask)), "causal masking not supported"
    x = np.ascontiguousarray(np.asarray(x, dtype=np.float32))
    Wk = np.asarray(Wk, dtype=np.float32)
    Wq = np.asarray(Wq, dtype=np.float32)
    Wv = np.asarray(Wv, dtype=np.float32)
    Wp = np.asarray(Wp, dtype=np.float32)

    c_scale = C ** (-0.5)
    d_scale = D ** (-0.5)
    wq_eff = Wq * (c_scale * d_scale)
    wk_eff = Wk * c_scale
    wv_eff = Wv * c_scale
    wp_eff = Wp * (INNER ** (-0.5))

    nc = _get_compiled()
    in_maps = []
    for core in range(8):
        b, half = divmod(core, 2)
        R = slice(half * RL, (half + 1) * RL)
        in_maps.append({
            "xT": _round_f32r(np.ascontiguousarray(x[b].T)),
            "wqT": _round_f32r(np.ascontiguousarray(wq_eff[R, :].T)),
            "wkT": _round_f32r(np.ascontiguousarray(wk_eff[R, :].T)),
            "wvT": _round_f32r(np.ascontiguousarray(wv_eff[R, :].T)),
            "wpT": _round_f32r(np.ascontiguousarray(wp_eff.T[R, :])),
        })

    global _last_in_maps
    _last_in_maps = in_maps
    res = run_bass_kernel_spmd(nc, in_maps, core_ids=list(range(8)))
    out = np.empty((B, T, O), dtype=np.float32)
    for b in range(B):
        out[b] = res.results[2 * b]["out"] + res.results[2 * b + 1]["out"]
    return out


# revision 29
# speedup vs baseline: 1.0084x; 1.0084x over previous
"""Multi-head attention (B=4, T=2048, H=8, D=64, C=512) on 8 NeuronCores.

Sharding: core c -> batch b = c//2, head group c%2 (4 heads = 256 inner rows).
Each core computes attention for its 4 heads plus the *partial* output
projection over its 256 inner rows; the host sums the two partials per batch
(the unshard of the contraction-sharded inner dimension).

Per-core dataflow (all fp32):
  qT,kT [128, 2, T] SBUF   pair-packed: partition = (h%2)*64 + d, dim1 = h//2
  v     [128, 16, 4, 128]  natural [keys, *] per head; 64 v-columns plus a
                           64-wide ones block. Even heads: [v | ones]; odd
                           heads: [ones | v]. The PV matmul then yields the
                           attention output on the partitions where the final
                           projection wants it, and the softmax denominator
                           broadcast over the other 64 partitions -- both for
                           free, since matmul cost is N (stream) cycles.
  scores: S^T per (pair, i-chunk, key-block) -> PSUM [128,512]; the two heads
          of a pair run as concurrent 64-row PE tiles (auto tile_position from
          base partitions 0/64). exp on ScalarE evacuates PSUM -> panel.
  PV:     po[128,512] accumulated over 16 key-blocks; recip of the denominator
          half, SBUF->SBUF DMA to the numerator partitions (DVE has no
          cross-lane path), then one fused multiply evacuates + normalizes.
"""

import numpy as np

B, T, C = 4, 2048, 512
H, D = 8, 64
INNER = H * D  # 512
O = 512
P = 128
IC = 512            # i-chunk (query) width
N_IC = T // IC      # 4
N_JB = T // P       # 16 key blocks
JBH = N_JB // 2     # key blocks per panel half
KC = C // P         # 4 contraction chunks
RL = 256            # local inner rows per core (4 heads)

_compiled = None
_last_in_maps = None


def _build():
    import concourse.bacc as bacc
    import concourse.tile as tile
    import concourse.mybir as mybir

    F32 = mybir.dt.float32
    F32R = mybir.dt.float32r
    Exp = mybir.ActivationFunctionType.Exp

    nc = bacc.Bacc("TRN2", target_bir_lowering=False, debug=False, num_devices=8)

    xT_d = nc.dram_tensor("xT", [C, T], F32R, kind="ExternalInput")
    wqT_d = nc.dram_tensor("wqT", [C, RL], F32R, kind="ExternalInput")
    wkT_d = nc.dram_tensor("wkT", [C, RL], F32R, kind="ExternalInput")
    wvT_d = nc.dram_tensor("wvT", [C, RL], F32R, kind="ExternalInput")
    wpT_d = nc.dram_tensor("wpT", [RL, O], F32R, kind="ExternalInput")
    out_d = nc.dram_tensor("out", [T, O], F32, kind="ExternalOutput")

    JBQ = 4  # key-blocks per panel quarter

    with tile.TileContext(nc) as tc:
        with (
            tc.tile_pool(name="persist", bufs=1) as persist,
            tc.tile_pool(name="rcp", bufs=2) as rcpool,
            tc.tile_pool(name="evac", bufs=2) as evac,
        ):
            wq = persist.tile([P, KC, RL], F32R, tag="wq")
            wk = persist.tile([P, KC, RL], F32R, tag="wk")
            wv = persist.tile([P, KC, RL], F32R, tag="wv")
            wp = persist.tile([P, 2, O], F32R, tag="wp")
            nc.sync.dma_start(wk[:], wkT_d.ap().rearrange("(kc p) r -> p kc r", p=P))

            qT = persist.tile([P, 2, T], F32R, tag="qT")
            kT = persist.tile([P, 2, T], F32R, tag="kT")
            vsb = persist.tile([P, N_JB, 4, P], F32R, tag="v")
            outT = persist.tile([P, 2, T], F32R, tag="outT")

            with tc.tile_pool(name="xtp", bufs=1) as xtp:
                # chunked xT load so the K projection starts on chunk 0
                xt = xtp.tile([P, KC, T], F32R, tag="xt")
                xt_r = xT_d.ap().rearrange("(kc p) t -> p kc t", p=P)
                for kc in range(KC):
                    nc.sync.dma_start(xt[:, kc, :], xt_r[:, kc, :])
                nc.sync.dma_start(
                    wq[:], wqT_d.ap().rearrange("(kc p) r -> p kc r", p=P)
                )
                nc.sync.dma_start(
                    wv[:], wvT_d.ap().rearrange("(kc p) r -> p kc r", p=P)
                )
                nc.sync.dma_start(
                    wp[:], wpT_d.ap().rearrange("(rc p) o -> p rc o", p=P)
                )

                # prewarm the exp table while DMAs stream
                warm = xtp.tile([P, 8], F32, tag="warm")
                nc.vector.memset(warm[:], 0.0)
                nc.scalar.activation(warm[:], warm[:], Exp)

                # keep the PE busy during the xT load so it is at full clock
                # (and past the HAM ramp) when the projections start
                wmm0 = xtp.tile([P, 256], F32, tag="wmm0")
                nc.vector.memset(wmm0[:], 0.0)
                wmm = xtp.tile([P, 256], F32R, tag="wmm")
                nc.vector.tensor_copy(wmm[:], wmm0[:])

                # ---- K projection, kc-outer: 8 live PSUM groups ----
                with tc.tile_pool(name="pk", bufs=5, space="PSUM") as pk:
                    wps = pk.tile([P, IC], F32, tag="kp", name="wps")
                    # only the first head-pair's K rows upfront (chunk order
                    # is pair-major, so rows 128:256 aren't needed until the
                    # 5th chunk and are projected inside chunk 1's PV loop)
                    kps = [
                        pk.tile([P, IC], F32, tag="kp", name=f"kp{g}")
                        for g in range(N_IC)
                    ]
                    for kc in range(KC):
                        for _ in range(20 if kc < KC - 1 else 0):
                            nc.tensor.matmul(wps[:, 0:256], lhsT=wmm[:, 0:P],
                                             rhs=wmm[:], start=True, stop=True)
                        for ic in range(N_IC):
                            nc.tensor.matmul(
                                kps[ic][:],
                                lhsT=wk[:, kc, 0:P],
                                rhs=xt[:, kc, ic * IC:(ic + 1) * IC],
                                start=(kc == 0), stop=(kc == KC - 1),
                            )
                    # evac the block chunk-0 scores need first, then squeeze
                    # chunk-0's Q projection in before the remaining evacs so
                    # the first exp isn't stuck behind them in the DVE queue
                    nc.vector.tensor_copy(kT[:, 0, 0:IC], kps[0][:])
                    qp0 = pk.tile([P, IC], F32, tag="kp", name="qp0")
                    for kc in range(KC):
                        nc.tensor.matmul(
                            qp0[:],
                            lhsT=wq[:, kc, 0:P],
                            rhs=xt[:, kc, 0:IC],
                            start=(kc == 0), stop=(kc == KC - 1),
                        )
                    nc.vector.tensor_copy(qT[:, 0, 0:IC], qp0[:])
                    for ic in range(1, N_IC):
                        nc.vector.tensor_copy(
                            kT[:, 0, ic * IC:(ic + 1) * IC], kps[ic][:]
                        )

                with (
                    tc.tile_pool(name="psA", bufs=2, space="PSUM") as psA,
                    tc.tile_pool(name="psB", bufs=4, space="PSUM") as psB,
                ):
                    # ones blocks of v (V projection itself is emitted after
                    # the first scores batch so ACT starts as early as possible)
                    ones_st = xtp.tile([P, N_JB, 64], F32, tag="ones")
                    nc.vector.memset(ones_st[:], 1.0)
                    ones_b = ones_st[:, :, None, :].to_broadcast(
                        (P, N_JB, 2, 64)
                    )
                    nc.vector.tensor_copy(vsb[:, :, 0:2, 64:128], ones_b)
                    nc.vector.tensor_copy(vsb[:, :, 2:4, 0:64], ones_b)

                    def k_rb1_group(ic):
                        kp = psB.tile([P, IC], F32, tag="o", name="kp1")
                        for kc in range(KC):
                            nc.tensor.matmul(
                                kp[:],
                                lhsT=wk[:, kc, P:2 * P],
                                rhs=xt[:, kc, ic * IC:(ic + 1) * IC],
                                start=(kc == 0), stop=(kc == KC - 1),
                            )
                        nc.vector.tensor_copy(
                            kT[:, 1, ic * IC:(ic + 1) * IC], kp[:]
                        )

                    def v_proj_group(jb):
                        ps = psB.tile([P, IC], F32, tag="o", name="vp")
                        for kc in range(KC):
                            nc.tensor.matmul(
                                ps[:, :RL],
                                lhsT=xt[:, kc, jb * P:(jb + 1) * P],
                                rhs=wv[:, kc, :],
                                start=(kc == 0), stop=(kc == KC - 1),
                            )
                        ps_r = ps[:, 0:RL].rearrange(
                            "p (e o d) -> p o e d", e=2, o=2
                        )
                        nc.vector.tensor_copy(vsb[:, jb, 0:2, 0:64],
                                              ps_r[:, 0])
                        nc.vector.tensor_copy(vsb[:, jb, 2:4, 64:128],
                                              ps_r[:, 1])

                    # ---- attention, i-chunk outer; final projection inline ----
                    # Pipelined emission: chunk c+1's scores quarters are
                    # interleaved into chunk c's PV segments (each quarter's
                    # panel slot is freed by the PV segment emitted just
                    # before it), so ScalarE never starves between chunks.
                    def q_proj(ic, pair):
                        icsl = slice(ic * IC, (ic + 1) * IC)
                        qp = psB.tile([P, IC], F32, tag="o", name="qp")
                        for kc in range(KC):
                            nc.tensor.matmul(
                                qp[:],
                                lhsT=wq[:, kc, pair * P:(pair + 1) * P],
                                rhs=xt[:, kc, icsl],
                                start=(kc == 0), stop=(kc == KC - 1),
                            )
                        nc.vector.tensor_copy(qT[:, pair, icsl], qp[:])

                    with tc.tile_pool(name="panels", bufs=4) as panels:
                        chunks = [(ic, pair) for pair in range(2)
                                  for ic in range(N_IC)]
                        NQ = N_JB // JBQ  # quarters per chunk
                        state = {}

                        def scores_quarter(idx):
                            ic, pair = chunks[idx]
                            icsl = slice(ic * IC, (ic + 1) * IC)
                            st = state.setdefault(idx, [])
                            quarter = len(st)
                            pan = panels.tile([P, JBQ, 2, IC], F32R, tag="pan")
                            st.append(pan)
                            for j4 in range(JBQ):
                                jb = quarter * JBQ + j4
                                ps = psA.tile([P, 2, IC], F32, tag="s")
                                for hh in range(2):
                                    nc.tensor.matmul(
                                        ps[:, hh, :],
                                        lhsT=kT[hh * 64:(hh + 1) * 64, pair,
                                                jb * P:(jb + 1) * P],
                                        rhs=qT[hh * 64:(hh + 1) * 64, pair,
                                               icsl],
                                        start=True, stop=True,
                                        skip_group_check=True,
                                    )
                                nc.scalar.activation(pan[:, j4, :, :], ps[:],
                                                     Exp)

                        for quarter in range(NQ):
                            scores_quarter(0)

                        for idx, (ic, pair) in enumerate(chunks):
                            icsl = slice(ic * IC, (ic + 1) * IC)
                            quarters = state[idx]
                            pos = []
                            for hh in range(2):
                                po = psB.tile([P, IC], F32, tag="o",
                                              name=f"po{hh}")
                                pos.append(po)
                            if idx + 1 < len(chunks):
                                q_proj(*chunks[idx + 1])
                            for quarter in range(NQ):
                                if idx == 1:
                                    k_rb1_group(quarter)
                                for j4 in range(JBQ):
                                    jb = quarter * JBQ + j4
                                    if idx == 0:
                                        v_proj_group(jb)
                                    for hh in range(2):
                                        nc.tensor.matmul(
                                            pos[hh][:],
                                            lhsT=vsb[:, jb, hh * 2 + pair, :],
                                            rhs=quarters[quarter][:, j4, hh, :],
                                            start=(jb == 0),
                                            stop=(jb == N_JB - 1),
                                        )
                                # chunk idx+1's scores quarter reuses the
                                # panel slot the PV segment above released
                                if idx + 1 < len(chunks):
                                    scores_quarter(idx + 1)
                            sls = [(slice(0, 64), slice(64, 128)),
                                   (slice(64, 128), slice(0, 64))]
                            rcs = []
                            for hh in range(2):
                                num_sl, den_sl = sls[hh]
                                rc = rcpool.tile([P, IC], F32, tag="rc")
                                rcs.append(rc)
                                nc.vector.reciprocal(rc[den_sl, :],
                                                     pos[hh][den_sl, :])
                            for hh in range(2):
                                num_sl, den_sl = sls[hh]
                                nc.sync.dma_start(rcs[hh][num_sl, :],
                                                  rcs[hh][den_sl, :])
                            for hh in range(2):
                                num_sl, den_sl = sls[hh]
                                nc.vector.tensor_mul(
                                    outT[num_sl, pair, icsl],
                                    pos[hh][num_sl, :],
                                    rcs[hh][num_sl, :],
                                )

                            if pair == 1:
                                # output projection for this chunk's i-blocks
                                for ib in range(ic * IC // P,
                                                (ic + 1) * IC // P):
                                    fp = psB.tile([P, IC], F32, tag="o",
                                                  name="fp")
                                    for pr in range(2):
                                        nc.tensor.matmul(
                                            fp[:],
                                            lhsT=outT[:, pr,
                                                      ib * P:(ib + 1) * P],
                                            rhs=wp[:, pr, :],
                                            start=(pr == 0), stop=(pr == 1),
                                        )
                                    ev = evac.tile([P, O], F32, tag="ev")
                                    if ic == N_IC - 1 and ib % 2 == 0:
                                        nc.scalar.copy(ev[:], fp[:])
                                    else:
                                        nc.vector.tensor_copy(ev[:], fp[:])
                                    nc.sync.dma_start(
                                        out_d[ib * P:(ib + 1) * P, :], ev[:]
                                    )

    nc.compile()
    return nc


def _get_compiled():
    global _compiled
    if _compiled is None:
        _compiled = _build()
    return _compiled


def _round_f32r(a):
    """Round fp32 to the FP32R-representable set: exact bf16 hi + bf16 lo."""
    import ml_dtypes

    a = np.asarray(a, dtype=np.float32)
    hi = a.astype(ml_dtypes.bfloat16).astype(np.float32)
    lo = (a - hi).astype(ml_dtypes.bfloat16).astype(np.float32)
    return hi + lo


def kernel(x, Wk, Wq, Wv, Wp, causal_mask):
    from concourse.bass_utils import run_bass_kernel_spmd

    assert not int(np.asarray(causal_mask)), "causal masking not supported"
    x = np.ascontiguousarray(np.asarray(x, dtype=np.float32))
    Wk = np.asarray(Wk, dtype=np.float32)
    Wq = np.asarray(Wq, dtype=np.float32)
    Wv = np.asarray(Wv, dtype=np.float32)
    Wp = np.asarray(Wp, dtype=np.float32)

    c_scale = C ** (-0.5)
    d_scale = D ** (-0.5)
    wq_eff = Wq * (c_scale * d_scale)
    wk_eff = Wk * c_scale
    wv_eff = Wv * c_scale
    wp_eff = Wp * (INNER ** (-0.5))

    nc = _get_compiled()
    in_maps = []
    for core in range(8):
        b, half = divmod(core, 2)
        R = slice(half * RL, (half + 1) * RL)
        in_maps.append({
            "xT": _round_f32r(np.ascontiguousarray(x[b].T)),
            "wqT": _round_f32r(np.ascontiguousarray(wq_eff[R, :].T)),
            "wkT": _round_f32r(np.ascontiguousarray(wk_eff[R, :].T)),
            "wvT": _round_f32r(np.ascontiguousarray(wv_eff[R, :].T)),
            "wpT": _round_f32r(np.ascontiguousarray(wp_eff.T[R, :])),
        })

    global _last_in_maps
    _last_in_maps = in_maps
    res = run_bass_kernel_spmd(nc, in_maps, core_ids=list(range(8)))
    out = np.empty((B, T, O), dtype=np.float32)
    for b in range(B):
        out[b] = res.results[2 * b]["out"] + res.results[2 * b + 1]["out"]
    return out


# revision 30
# speedup vs baseline: 1.0126x; 1.0041x over previous
"""Multi-head attention (B=4, T=2048, H=8, D=64, C=512) on 8 NeuronCores.

Sharding: core c -> batch b = c//2, head group c%2 (4 heads = 256 inner rows).
Each core computes attention for its 4 heads plus the *partial* output
projection over its 256 inner rows; the host sums the two partials per batch
(the unshard of the contraction-sharded inner dimension).

Per-core dataflow (all fp32):
  qT,kT [128, 2, T] SBUF   pair-packed: partition = (h%2)*64 + d, dim1 = h//2
  v     [128, 16, 4, 128]  natural [keys, *] per head; 64 v-columns plus a
                           64-wide ones block. Even heads: [v | ones]; odd
                           heads: [ones | v]. The PV matmul then yields the
                           attention output on the partitions where the final
                           projection wants it, and the softmax denominator
                           broadcast over the other 64 partitions -- both for
                           free, since matmul cost is N (stream) cycles.
  scores: S^T per (pair, i-chunk, key-block) -> PSUM [128,512]; the two heads
          of a pair run as concurrent 64-row PE tiles (auto tile_position from
          base partitions 0/64). exp on ScalarE evacuates PSUM -> panel.
  PV:     po[128,512] accumulated over 16 key-blocks; recip of the denominator
          half, SBUF->SBUF DMA to the numerator partitions (DVE has no
          cross-lane path), then one fused multiply evacuates + normalizes.
"""

import numpy as np

B, T, C = 4, 2048, 512
H, D = 8, 64
INNER = H * D  # 512
O = 512
P = 128
IC = 512            # i-chunk (query) width
N_IC = T // IC      # 4
N_JB = T // P       # 16 key blocks
JBH = N_JB // 2     # key blocks per panel half
KC = C // P         # 4 contraction chunks
RL = 256            # local inner rows per core (4 heads)

_compiled = None
_last_in_maps = None


def _build():
    import concourse.bacc as bacc
    import concourse.tile as tile
    import concourse.mybir as mybir

    F32 = mybir.dt.float32
    F32R = mybir.dt.float32r
    Exp = mybir.ActivationFunctionType.Exp

    nc = bacc.Bacc("TRN2", target_bir_lowering=False, debug=False, num_devices=8)

    xT_d = nc.dram_tensor("xT", [C, T], F32R, kind="ExternalInput")
    wqT_d = nc.dram_tensor("wqT", [C, RL], F32R, kind="ExternalInput")
    wkT_d = nc.dram_tensor("wkT", [C, RL], F32R, kind="ExternalInput")
    wvT_d = nc.dram_tensor("wvT", [C, RL], F32R, kind="ExternalInput")
    wpT_d = nc.dram_tensor("wpT", [RL, O], F32R, kind="ExternalInput")
    out_d = nc.dram_tensor("out", [T, O], F32, kind="ExternalOutput")

    JBQ = 4  # key-blocks per panel quarter

    with tile.TileContext(nc) as tc:
        with (
            tc.tile_pool(name="persist", bufs=1) as persist,
            tc.tile_pool(name="rcp", bufs=2) as rcpool,
            tc.tile_pool(name="evac", bufs=3) as evac,
        ):
            wq = persist.tile([P, KC, RL], F32R, tag="wq")
            wk = persist.tile([P, KC, RL], F32R, tag="wk")
            wv = persist.tile([P, KC, RL], F32R, tag="wv")
            wp = persist.tile([P, 2, O], F32R, tag="wp")
            nc.sync.dma_start(wk[:], wkT_d.ap().rearrange("(kc p) r -> p kc r", p=P))

            qT = persist.tile([P, 2, T], F32R, tag="qT")
            kT = persist.tile([P, 2, T], F32R, tag="kT")
            vsb = persist.tile([P, N_JB, 4, P], F32R, tag="v")
            outT = persist.tile([P, 2, T], F32R, tag="outT")

            with tc.tile_pool(name="xtp", bufs=1) as xtp:
                # chunked xT load so the K projection starts on chunk 0
                xt = xtp.tile([P, KC, T], F32R, tag="xt")
                xt_r = xT_d.ap().rearrange("(kc p) t -> p kc t", p=P)
                for kc in range(KC):
                    nc.sync.dma_start(xt[:, kc, :], xt_r[:, kc, :])
                nc.sync.dma_start(
                    wq[:], wqT_d.ap().rearrange("(kc p) r -> p kc r", p=P)
                )
                nc.sync.dma_start(
                    wv[:], wvT_d.ap().rearrange("(kc p) r -> p kc r", p=P)
                )
                nc.sync.dma_start(
                    wp[:], wpT_d.ap().rearrange("(rc p) o -> p rc o", p=P)
                )

                # prewarm the exp table while DMAs stream
                warm = xtp.tile([P, 8], F32, tag="warm")
                nc.vector.memset(warm[:], 0.0)
                nc.scalar.activation(warm[:], warm[:], Exp)

                # keep the PE busy during the xT load so it is at full clock
                # (and past the HAM ramp) when the projections start
                wmm0 = xtp.tile([P, 256], F32, tag="wmm0")
                nc.vector.memset(wmm0[:], 0.0)
                wmm = xtp.tile([P, 256], F32R, tag="wmm")
                nc.vector.tensor_copy(wmm[:], wmm0[:])

                # ---- K projection, kc-outer: 8 live PSUM groups ----
                with tc.tile_pool(name="pk", bufs=5, space="PSUM") as pk:
                    wps = pk.tile([P, IC], F32, tag="kp", name="wps")
                    # only the first head-pair's K rows upfront (chunk order
                    # is pair-major, so rows 128:256 aren't needed until the
                    # 5th chunk and are projected inside chunk 1's PV loop)
                    kps = [
                        pk.tile([P, IC], F32, tag="kp", name=f"kp{g}")
                        for g in range(N_IC)
                    ]
                    for kc in range(KC):
                        for _ in range(20 if kc < KC - 1 else 0):
                            nc.tensor.matmul(wps[:, 0:256], lhsT=wmm[:, 0:P],
                                             rhs=wmm[:], start=True, stop=True)
                        for ic in range(N_IC):
                            nc.tensor.matmul(
                                kps[ic][:],
                                lhsT=wk[:, kc, 0:P],
                                rhs=xt[:, kc, ic * IC:(ic + 1) * IC],
                                start=(kc == 0), stop=(kc == KC - 1),
                            )
                    # evac the block chunk-0 scores need first, then squeeze
                    # chunk-0's Q projection in before the remaining evacs so
                    # the first exp isn't stuck behind them in the DVE queue
                    nc.vector.tensor_copy(kT[:, 0, 0:IC], kps[0][:])
                    qp0 = pk.tile([P, IC], F32, tag="kp", name="qp0")
                    for kc in range(KC):
                        nc.tensor.matmul(
                            qp0[:],
                            lhsT=wq[:, kc, 0:P],
                            rhs=xt[:, kc, 0:IC],
                            start=(kc == 0), stop=(kc == KC - 1),
                        )
                    nc.vector.tensor_copy(qT[:, 0, 0:IC], qp0[:])
                    for ic in range(1, N_IC):
                        nc.vector.tensor_copy(
                            kT[:, 0, ic * IC:(ic + 1) * IC], kps[ic][:]
                        )

                with (
                    tc.tile_pool(name="psA", bufs=2, space="PSUM") as psA,
                    tc.tile_pool(name="psB", bufs=4, space="PSUM") as psB,
                ):
                    # ones blocks of v (V projection itself is emitted after
                    # the first scores batch so ACT starts as early as possible)
                    ones_st = xtp.tile([P, N_JB // 2, 64], F32, tag="ones")
                    nc.vector.memset(ones_st[:], 1.0)
                    ones_b = ones_st[:, :, None, :].to_broadcast(
                        (P, N_JB // 2, 2, 64)
                    )
                    for jh in range(2):
                        jsl = slice(jh * (N_JB // 2), (jh + 1) * (N_JB // 2))
                        nc.vector.tensor_copy(vsb[:, jsl, 0:2, 64:128], ones_b)
                        nc.vector.tensor_copy(vsb[:, jsl, 2:4, 0:64], ones_b)

                    def k_rb1_group(ic):
                        kp = psB.tile([P, IC], F32, tag="o", name="kp1")
                        for kc in range(KC):
                            nc.tensor.matmul(
                                kp[:],
                                lhsT=wk[:, kc, P:2 * P],
                                rhs=xt[:, kc, ic * IC:(ic + 1) * IC],
                                start=(kc == 0), stop=(kc == KC - 1),
                            )
                        nc.vector.tensor_copy(
                            kT[:, 1, ic * IC:(ic + 1) * IC], kp[:]
                        )

                    def v_proj_group(jb):
                        ps = psB.tile([P, IC], F32, tag="o", name="vp")
                        for kc in range(KC):
                            nc.tensor.matmul(
                                ps[:, :RL],
                                lhsT=xt[:, kc, jb * P:(jb + 1) * P],
                                rhs=wv[:, kc, :],
                                start=(kc == 0), stop=(kc == KC - 1),
                            )
                        ps_r = ps[:, 0:RL].rearrange(
                            "p (e o d) -> p o e d", e=2, o=2
                        )
                        nc.vector.tensor_copy(vsb[:, jb, 0:2, 0:64],
                                              ps_r[:, 0])
                        nc.vector.tensor_copy(vsb[:, jb, 2:4, 64:128],
                                              ps_r[:, 1])

                    # ---- attention, i-chunk outer; final projection inline ----
                    # Pipelined emission: chunk c+1's scores quarters are
                    # interleaved into chunk c's PV segments (each quarter's
                    # panel slot is freed by the PV segment emitted just
                    # before it), so ScalarE never starves between chunks.
                    def q_proj(ic, pair):
                        icsl = slice(ic * IC, (ic + 1) * IC)
                        qp = psB.tile([P, IC], F32, tag="o", name="qp")
                        for kc in range(KC):
                            nc.tensor.matmul(
                                qp[:],
                                lhsT=wq[:, kc, pair * P:(pair + 1) * P],
                                rhs=xt[:, kc, icsl],
                                start=(kc == 0), stop=(kc == KC - 1),
                            )
                        nc.vector.tensor_copy(qT[:, pair, icsl], qp[:])

                    with tc.tile_pool(name="panels", bufs=4) as panels:
                        chunks = [(ic, pair) for pair in range(2)
                                  for ic in range(N_IC)]
                        NQ = N_JB // JBQ  # quarters per chunk
                        state = {}

                        def scores_quarter(idx):
                            ic, pair = chunks[idx]
                            icsl = slice(ic * IC, (ic + 1) * IC)
                            st = state.setdefault(idx, [])
                            quarter = len(st)
                            pan = panels.tile([P, JBQ, 2, IC], F32R, tag="pan")
                            st.append(pan)
                            for j4 in range(JBQ):
                                jb = quarter * JBQ + j4
                                ps = psA.tile([P, 2, IC], F32, tag="s")
                                for hh in range(2):
                                    nc.tensor.matmul(
                                        ps[:, hh, :],
                                        lhsT=kT[hh * 64:(hh + 1) * 64, pair,
                                                jb * P:(jb + 1) * P],
                                        rhs=qT[hh * 64:(hh + 1) * 64, pair,
                                               icsl],
                                        start=True, stop=True,
                                        skip_group_check=True,
                                    )
                                nc.scalar.activation(pan[:, j4, :, :], ps[:],
                                                     Exp)

                        for quarter in range(NQ):
                            scores_quarter(0)

                        for idx, (ic, pair) in enumerate(chunks):
                            icsl = slice(ic * IC, (ic + 1) * IC)
                            quarters = state[idx]
                            pos = []
                            for hh in range(2):
                                po = psB.tile([P, IC], F32, tag="o",
                                              name=f"po{hh}")
                                pos.append(po)
                            if idx + 1 < len(chunks):
                                q_proj(*chunks[idx + 1])
                            for quarter in range(NQ):
                                if idx == 1:
                                    k_rb1_group(quarter)
                                for j4 in range(JBQ):
                                    jb = quarter * JBQ + j4
                                    if idx == 0:
                                        v_proj_group(jb)
                                    for hh in range(2):
                                        nc.tensor.matmul(
                                            pos[hh][:],
                                            lhsT=vsb[:, jb, hh * 2 + pair, :],
                                            rhs=quarters[quarter][:, j4, hh, :],
                                            start=(jb == 0),
                                            stop=(jb == N_JB - 1),
                                        )
                                # chunk idx+1's scores quarter reuses the
                                # panel slot the PV segment above released
                                if idx + 1 < len(chunks):
                                    scores_quarter(idx + 1)
                            sls = [(slice(0, 64), slice(64, 128)),
                                   (slice(64, 128), slice(0, 64))]
                            rcs = []
                            for hh in range(2):
                                num_sl, den_sl = sls[hh]
                                rc = rcpool.tile([P, IC], F32, tag="rc")
                                rcs.append(rc)
                                nc.vector.reciprocal(rc[den_sl, :],
                                                     pos[hh][den_sl, :])
                            for hh in range(2):
                                num_sl, den_sl = sls[hh]
                                nc.sync.dma_start(rcs[hh][num_sl, :],
                                                  rcs[hh][den_sl, :])
                            for hh in range(2):
                                num_sl, den_sl = sls[hh]
                                nc.vector.tensor_mul(
                                    outT[num_sl, pair, icsl],
                                    pos[hh][num_sl, :],
                                    rcs[hh][num_sl, :],
                                )

                            if pair == 1:
                                # output projection for this chunk's i-blocks
                                for ib in range(ic * IC // P,
                                                (ic + 1) * IC // P):
                                    fp = psB.tile([P, IC], F32, tag="o",
                                                  name="fp")
                                    for pr in range(2):
                                        nc.tensor.matmul(
                                            fp[:],
                                            lhsT=outT[:, pr,
                                                      ib * P:(ib + 1) * P],
                                            rhs=wp[:, pr, :],
                                            start=(pr == 0), stop=(pr == 1),
                                        )
                                    ev = evac.tile([P, O], F32, tag="ev")
                                    if ic == N_IC - 1 and ib % 2 == 0:
                                        nc.scalar.copy(ev[:], fp[:])
                                    else:
                                        nc.vector.tensor_copy(ev[:], fp[:])
                                    nc.sync.dma_start(
                                        out_d[ib * P:(ib + 1) * P, :], ev[:]
                                    )

    nc.compile()
    return nc


def _get_compiled():
    global _compiled
    if _compiled is None:
        _compiled = _build()
    return _compiled


def _round_f32r(a):
    """Round fp32 to the FP32R-representable set: exact bf16 hi + bf16 lo."""
    import ml_dtypes

    a = np.asarray(a, dtype=np.float32)
    hi = a.astype(ml_dtypes.bfloat16).astype(np.float32)
    lo = (a - hi).astype(ml_dtypes.bfloat16).astype(np.float32)
    return hi + lo


def kernel(x, Wk, Wq, Wv, Wp, causal_mask):
    from concourse.bass_utils import run_bass_kernel_spmd

    assert not int(np.asarray(causal_mask)), "causal masking not supported"
    x = np.ascontiguousarray(np.asarray(x, dtype=np.float32))
    Wk = np.asarray(Wk, dtype=np.float32)
    Wq = np.asarray(Wq, dtype=np.float32)
    Wv = np.asarray(Wv, dtype=np.float32)
    Wp = np.asarray(Wp, dtype=np.float32)

    c_scale = C ** (-0.5)
    d_scale = D ** (-0.5)
    wq_eff = Wq * (c_scale * d_scale)
    wk_eff = Wk * c_scale
    wv_eff = Wv * c_scale
    wp_eff = Wp * (INNER ** (-0.5))

    nc = _get_compiled()
    in_maps = []
    for core in range(8):
        b, half = divmod(core, 2)
        R = slice(half * RL, (half + 1) * RL)
        in_maps.append({
            "xT": _round_f32r(np.ascontiguousarray(x[b].T)),
            "wqT": _round_f32r(np.ascontiguousarray(wq_eff[R, :].T)),
            "wkT": _round_f32r(np.ascontiguousarray(wk_eff[R, :].T)),
            "wvT": _round_f32r(np.ascontiguousarray(wv_eff[R, :].T)),
            "wpT": _round_f32r(np.ascontiguousarray(wp_eff.T[R, :])),
        })

    global _last_in_maps
    _last_in_maps = in_maps
    res = run_bass_kernel_spmd(nc, in_maps, core_ids=list(range(8)))
    out = np.empty((B, T, O), dtype=np.float32)
    for b in range(B):
        out[b] = res.results[2 * b]["out"] + res.results[2 * b + 1]["out"]
    return out
